# revision 1
# baseline (speedup 1.0000x reference)
"""CrossGatingBlock Trainium2 kernel builder (one sample per core)."""
import numpy as np
import concourse.bass as bass
import concourse.mybir as mybir
from concourse import tile

fp32 = mybir.dt.float32
fp32r = mybir.dt.float32r
bf16 = mybir.dt.bfloat16
AF = mybir.ActivationFunctionType
ALU = mybir.AluOpType
AX = mybir.AxisListType

C = 128
HW = 128 * 128
NM = float(C * HW)
EPS = 1e-5
NT = 16


def split_waits(nc, limit=1):
    for b in nc.m.functions[0].blocks:
        insts = list(b.instructions)
        newlist = []
        for inst in insts:
            si = inst.sync_info
            waits = list(si.on_wait) if si is not None and si.on_wait else []
            if len(waits) > limit:
                extra, keep = waits[:-limit], waits[-limit:]
                for s in range(0, len(extra), limit):
                    grp = extra[s:s + limit]
                    nop = mybir.InstNoOp(
                        name=nc.get_next_instruction_name(),
                        engine=inst.engine,
                        ins=[], outs=[],
                        sync_info=mybir.SyncInfo(on_wait=grp, on_update=[]),
                    )
                    newlist.append(nop)
                del si.on_wait[:]
                si.on_wait.extend(keep)
            newlist.append(inst)
        del b.instructions[:]
        b.instructions.extend(newlist)


def build_nc(debug=False):
    nc = bass.Bass("TRN2")
    P = {}

    def par(name, shape, dt):
        P[name] = nc.declare_dram_parameter(name, list(shape), dt, isOutput=False)

    par("X", (C, HW), fp32r)
    par("Y", (2, C, 4096), fp32r)
    par("WCT", (C, 2, 4, C), fp32r)
    for w in ("WC1", "WC2", "WL1", "WL2", "WL3", "WL4"):
        par(w, (C, C), fp32r)
    for g in ("1", "2"):
        par(f"G{g}W1", (C, 2, C), fp32r)
        par(f"G{g}W2", (C, 2, 256), fp32r)
        par(f"G{g}W3", (C, 2, 256), fp32r)
        par(f"G{g}W4U", (C, C), bf16)
        par(f"G{g}W4V", (C, C), bf16)
        par(f"F{g}UL", (2, C), fp32r)
        par(f"F{g}UR", (2, 256), fp32r)
        par(f"F{g}VL", (1, C), fp32r)
        par(f"F{g}VR", (1, 256), fp32r)
        par(f"G{g}B1", (C, 2), fp32)
        par(f"CSG{g}W1", (C, 2), fp32)
    for b in ("BT", "BC1", "BC2", "BL1", "BL2", "BL3", "BL4", "CSWL1", "CSWL2"):
        par(b, (C, 1), fp32)
    par("ONESR", (1, C), fp32)
    par("ONESC", (C, 1), fp32)

    XO = nc.declare_dram_parameter("XO", [C, HW], fp32, isOutput=True)
    YO = nc.declare_dram_parameter("YO", [C, HW], fp32, isOutput=True)

    DBG = {}
    if debug:
        for d in ("DA", "DB", "DX1", "DGX", "DGY", "DV1", "DY1"):
            DBG[d] = nc.declare_dram_parameter(d, [C, HW], fp32, isOutput=True)
    A_SP = nc.dram_tensor("A_SP", [C, HW], fp32r)
    B_SP = nc.dram_tensor("B_SP", [C, HW], fp32r)
    X1_SP = nc.dram_tensor("X1_SP", [C, HW], fp32r)
    YO_SC = nc.dram_tensor("YO_SC", [C, HW], fp32r)

    with tile.TileContext(nc) as tc:
        wpool = tc.alloc_tile_pool(name="weights", bufs=1, side="left")
        spool = tc.alloc_tile_pool(name="smalls", bufs=1, side="left")
        evt = tc.alloc_tile_pool(name="evt", bufs=3, side="left")
        ps_big = tc.alloc_tile_pool(name="ps_big", bufs=2, space="PSUM", side="left")
        ps_mm1 = tc.alloc_tile_pool(name="ps_mm1", bufs=2, space="PSUM", side="left")
        ps_mm2 = tc.alloc_tile_pool(name="ps_mm2", bufs=2, space="PSUM", side="right")

        W = {}
        for name, h in P.items():
            if name in ("X", "Y"):
                continue
            t = wpool.tile(list(h.shape), h.dtype, tag=f"w_{name}")
            nc.sync.dma_start(t[:], h[:])
            W[name] = t

        eps_t = spool.tile([1, 1], fp32, tag="eps")
        nc.vector.memset(eps_t[:], EPS)

        # ---------- helpers ----------
        def ln_scalars(sum_t, sq_t, k, tag):
            s1 = spool.tile([C, 1], fp32, tag=f"{tag}_s1")
            s2 = spool.tile([C, 1], fp32, tag=f"{tag}_s2")
            nc.vector.tensor_reduce(s1[:], sum_t[:, 0:k], axis=AX.X, op=ALU.add)
            nc.vector.tensor_reduce(s2[:], sq_t[:, 0:k], axis=AX.X, op=ALU.add)
            pst = ps_mm2.tile([1, 2], fp32, tag="ps_mm2")
            nc.tensor.matmul(pst[:, 0:1], s1[:], W["ONESC"][:], start=True, stop=True)
            nc.tensor.matmul(pst[:, 1:2], s2[:], W["ONESC"][:], start=True, stop=True)
            sc = spool.tile([1, 2], fp32, tag=f"{tag}_sc")
            nc.vector.tensor_copy(sc[:], pst[:])
            mean = spool.tile([1, 1], fp32, tag=f"{tag}_mean")
            ex2 = spool.tile([1, 1], fp32, tag=f"{tag}_ex2")
            nc.scalar.mul(mean[:], sc[:, 0:1], 1.0 / NM)
            nc.scalar.mul(ex2[:], sc[:, 1:2], 1.0 / NM)
            m2 = spool.tile([1, 1], fp32, tag=f"{tag}_m2")
            nc.scalar.square(m2[:], mean[:])
            var = spool.tile([1, 1], fp32, tag=f"{tag}_var")
            nc.vector.tensor_sub(var[:], ex2[:], m2[:])
            std = spool.tile([1, 1], fp32, tag=f"{tag}_std")
            nc.scalar.activation(std[:], var[:], AF.Sqrt, bias=eps_t[:])
            rstd = spool.tile([1, 1], fp32, tag=f"{tag}_rstd")
            nc.vector.reciprocal(rstd[:], std[:])
            nmr = spool.tile([1, 1], fp32, tag=f"{tag}_nmr")
            nc.vector.scalar_tensor_tensor(
                nmr[:], mean[:], -1.0, rstd[:], ALU.mult, ALU.mult)
            rn = spool.tile([1, 2], fp32, tag=f"{tag}_rn")
            nc.vector.tensor_copy(rn[:, 0:1], rstd[:])
            nc.vector.tensor_copy(rn[:, 1:2], nmr[:])
            psb = ps_mm2.tile([C, 2], fp32, tag="ps_mm2")
            nc.tensor.matmul(psb[:], W["ONESR"][:], rn[:], start=True, stop=True)
            rs_b = spool.tile([C, 2], fp32, tag=f"{tag}_rsb")
            nc.vector.tensor_copy(rs_b[:], psb[:])
            return rs_b

        def fold_bias(rs_b, cs_col, b_col, tag):
            bv = spool.tile([C, 1], fp32, tag=f"{tag}_bv")
            nc.vector.scalar_tensor_tensor(
                bv[:], cs_col, rs_b[:, 1:2], b_col, ALU.mult, ALU.add)
            return bv

        def chan_stage(rhs_fn, wt, bias_ap, func, out_t, stats=None, scale=1.0,
                       extra_dma=None):
            for i in range(NT):
                ps = ps_big.tile([C, 1024], fp32, tag="ps_big")
                rhs = rhs_fn(i)
                for j in range(2):
                    nc.tensor.matmul(ps[:, j * 512:(j + 1) * 512], wt,
                                     rhs[:, j * 512:(j + 1) * 512],
                                     start=True, stop=True)
                kw = {}
                if stats is not None:
                    kw["accum_out"] = stats[:, i:i + 1]
                nc.scalar.activation(out_t[:, i * 1024:(i + 1) * 1024], ps[:],
                                     func, bias=bias_ap, scale=scale, **kw)
                if extra_dma is not None:
                    nc.sync.dma_start(extra_dma[:, i * 1024:(i + 1) * 1024],
                                      out_t[:, i * 1024:(i + 1) * 1024])

        def square_stats(src_t, stats):
            for i in range(NT):
                scrap = evt.tile([C, 1024], fp32, tag="cw1024")
                nc.scalar.activation(scrap[:], src_t[:, i * 1024:(i + 1) * 1024],
                                     AF.Square, accum_out=stats[:, i:i + 1])

        def gating_w1_u(t_sb, g, rs_b, gx_t, v1_t):
            w1, w2 = W[f"G{g}W1"], W[f"G{g}W2"]
            w4u = W[f"G{g}W4U"]
            bv = [fold_bias(rs_b, W[f"CSG{g}W1"][:, j:j + 1],
                            W[f"G{g}B1"][:, j:j + 1], f"g{g}c{j}") for j in range(2)]
            scale = rs_b[:, 0:1]
            t_grid = t_sb[:].rearrange("c (gh fh gw fw) -> c fh fw gh gw",
                                       gh=16, fh=8, gw=16, fw=8)
            v_bm = v1_t[:].rearrange("c (gh1 gw1 qh qw) -> c qw qh gh1 gw1",
                                     gh1=8, gw1=8, qh=16, qw=16)
            gx_grid = gx_t[:].rearrange("c (gh fh gw fw) -> c fh fw gh gw",
                                        gh=16, fh=8, gw=16, fw=8)
            for fh in range(8):
                for fw0 in range(0, 8, 2):
                    ps = ps_big.tile([C, 2, 512], fp32, tag="ps_big")
                    rhs = t_grid[:, fh, fw0:fw0 + 2, :, :]
                    for j in range(2):
                        nc.tensor.matmul(ps[:, j, :], w1[:, j, :], rhs,
                                         start=True, stop=True)
                    u_t = evt.tile([C, 2, 256], bf16, tag="ut")
                    nc.scalar.activation(u_t[:], ps[:, 0, :], AF.Gelu,
                                         bias=bv[0][:], scale=scale)
                    ps_v = ps[:, 1, :].rearrange(
                        "c (p gh1 gh0 gw1 gw0) -> c gh0 gw0 p gh1 gw1",
                        p=2, gh1=8, gh0=2, gw1=8, gw0=2)
                    for gh0 in range(2):
                        for gw0 in range(2):
                            nc.scalar.activation(
                                v_bm[:, gw0 * 8 + fw0:gw0 * 8 + fw0 + 2,
                                     gh0 * 8 + fh, :, :],
                                ps_v[:, gh0, gw0, :, :, :], AF.Gelu,
                                bias=bv[1][:], scale=scale)
                    ps1 = ps_mm1.tile([C, 4, C], fp32, tag="ps_mm1")
                    for pi in range(2):
                        for gc in range(2):
                            nc.tensor.matmul(ps1[:, pi * 2 + gc, :],
                                             u_t[:, pi, gc * 128:(gc + 1) * 128],
                                             w4u[:], start=True, stop=True)
                    tt = evt.tile([C, 4, C], fp32r, tag="tt")
                    nc.scalar.activation(tt[:], ps1[:], AF.Copy)
                    for pi in range(2):
                        ps2 = ps_mm2.tile([C, 256], fp32, tag="ps_mm2")
                        for gc in range(2):
                            nc.tensor.matmul(ps2[:], tt[:, pi * 2 + gc, :],
                                             w2[:, gc, :], start=(gc == 0), stop=False)
                        nc.tensor.matmul(ps2[:], W[f"F{g}UL"][:], W[f"F{g}UR"][:],
                                         start=False, stop=True)
                        nc.scalar.activation(
                            gx_grid[:, fh, fw0 + pi, :, :], ps2[:], AF.Copy)

        def gating_v(g, gx_t, v1_t):
            w3 = W[f"G{g}W3"]
            w4v = W[f"G{g}W4V"]
            t_blk = v1_t[:].rearrange("c (b q) -> c b q", b=64)
            gx_blk = gx_t[:].rearrange("c (bh qh bw qw) -> c bh bw qh qw",
                                       bh=8, qh=16, bw=8, qw=16)
            for bh in range(8):
                for bw0 in range(0, 8, 2):
                    ps1 = ps_mm1.tile([C, 4, C], fp32, tag="ps_mm1")
                    for bi in range(2):
                        for qc in range(2):
                            lhs = t_blk[:, bh * 8 + bw0 + bi,
                                        qc * 128:(qc + 1) * 128]
                            nc.tensor.matmul(ps1[:, bi * 2 + qc, :], lhs, w4v[:],
                                             start=True, stop=True)
                    tt = evt.tile([C, 4, C], fp32r, tag="tt")
                    nc.scalar.activation(tt[:], ps1[:], AF.Copy)
                    for bi in range(2):
                        ps2 = ps_mm2.tile([C, 256], fp32, tag="ps_mm2")
                        for qc in range(2):
                            nc.tensor.matmul(ps2[:], tt[:, bi * 2 + qc, :],
                                             w3[:, qc, :], start=(qc == 0), stop=False)
                        nc.tensor.matmul(ps2[:], W[f"F{g}VL"][:], W[f"F{g}VR"][:],
                                         start=False, stop=True)
                        dst = gx_blk[:, bh, bw0 + bi, :, :]
                        nc.vector.scalar_tensor_tensor(
                            dst, ps2[:], 1.0, dst, ALU.mult, ALU.add)

        # ================= schedule =================
        # --- x head ---
        a_pool = tc.alloc_tile_pool(name="a", bufs=1, side="right")
        a_t = a_pool.tile([C, HW], fp32r, tag="a_t")
        xs_pool = tc.alloc_tile_pool(name="xs", bufs=1, side="left")
        x_s = xs_pool.tile([C, HW], fp32r, tag="x_s")
        nc.sync.dma_start(x_s[:], P["X"][:])
        st_a1 = spool.tile([C, NT], fp32, tag="st_a1")
        st_a2 = spool.tile([C, NT], fp32, tag="st_a2")
        chan_stage(lambda i: x_s[:, i * 1024:(i + 1) * 1024], W["WC1"][:],
                   W["BC1"][:], AF.Relu, a_t, stats=st_a1, extra_dma=A_SP)
        xs_pool.release()
        square_stats(a_t, st_a2)
        rs_a = ln_scalars(st_a1, st_a2, NT, "ln_a")
        bv_a = fold_bias(rs_a, W["CSWL1"][:], W["BL1"][:], "f_a")

        x1_pool = tc.alloc_tile_pool(name="x1", bufs=1, side="left")
        x1_t = x1_pool.tile([C, HW], fp32r, tag="x1_t")
        st_x1 = spool.tile([C, NT], fp32, tag="st_x1")
        st_x2 = spool.tile([C, NT], fp32, tag="st_x2")
        chan_stage(lambda i: a_t[:, i * 1024:(i + 1) * 1024], W["WL1"][:],
                   bv_a[:], AF.Gelu, x1_t, stats=st_x1, scale=rs_a[:, 0:1],
                   extra_dma=X1_SP)
        a_pool.release()
        square_stats(x1_t, st_x2)
        rs_x1 = ln_scalars(st_x1, st_x2, NT, "ln_x1")

        # --- g1 gating ---
        gx_pool = tc.alloc_tile_pool(name="gx", bufs=1, side="right")
        gx_t = gx_pool.tile([C, HW], bf16, tag="gx_t")
        v1_pool = tc.alloc_tile_pool(name="v1", bufs=1, side="right")
        v1_t = v1_pool.tile([C, HW], bf16, tag="v1_t")
        gating_w1_u(x1_t, "1", rs_x1, gx_t, v1_t)
        if debug:
            nc.sync.dma_start(DBG["DX1"][:].bitcast(fp32r), x1_t[:])
        x1_pool.release()
        gating_v("1", gx_t, v1_t)
        if debug:
            for i in range(NT):
                dt_ = evt.tile([C, 1024], fp32, tag="cw1024")
                nc.vector.tensor_copy(dt_[:], v1_t[:, i*1024:(i+1)*1024])
                nc.sync.dma_start(DBG["DV1"][:, i*1024:(i+1)*1024], dt_[:])
                dt2 = evt.tile([C, 1024], fp32, tag="cw1024")
                nc.vector.tensor_copy(dt2[:], gx_t[:, i*1024:(i+1)*1024])
                nc.sync.dma_start(DBG["DGX"][:, i*1024:(i+1)*1024], dt2[:])
        v1_pool.release()

        # --- y head: convT -> wc2 -> b ---
        ys_pool = tc.alloc_tile_pool(name="ys", bufs=1, side="left")
        y_s = ys_pool.tile([C, 2, 4096], fp32r, tag="y_s")
        for kc in range(2):
            nc.sync.dma_start(y_s[:, kc, :], P["Y"][kc])
        b_pool = tc.alloc_tile_pool(name="b", bufs=1, side="right")
        b_t = b_pool.tile([C, HW], fp32r, tag="b_t")
        st_b1 = spool.tile([C, NT], fp32, tag="st_b1")
        st_b2 = spool.tile([C, NT], fp32, tag="st_b2")
        for i in range(NT):
            psc = ps_big.tile([C, 4, 256], fp32, tag="ps_big")
            ycols = y_s[:, :, i * 256:(i + 1) * 256]
            for kl in range(4):
                for kc in range(2):
                    nc.tensor.matmul(psc[:, kl, :], W["WCT"][:, kc, kl, :],
                                     ycols[:, kc, :], start=(kc == 0), stop=(kc == 1))
            b0 = evt.tile([C, 1024], fp32r, tag="io1024")
            b0v = b0[:].rearrange("c (h k w l) -> c k l h w", h=4, k=2, w=64, l=2)
            for k in range(2):
                for l in range(2):
                    src = psc[:, k * 2 + l, :].rearrange("c (h w) -> c h w", h=4)
                    nc.scalar.activation(b0v[:, k, l, :, :], src, AF.Relu,
                                         bias=W["BT"][:])
            ps = ps_big.tile([C, 1024], fp32, tag="ps_big")
            for j in range(2):
                nc.tensor.matmul(ps[:, j * 512:(j + 1) * 512], W["WC2"][:],
                                 b0[:, j * 512:(j + 1) * 512], start=True, stop=True)
            nc.scalar.activation(b_t[:, i * 1024:(i + 1) * 1024], ps[:], AF.Relu,
                                 bias=W["BC2"][:], accum_out=st_b1[:, i:i + 1])
            nc.sync.dma_start(B_SP[:, i * 1024:(i + 1) * 1024],
                              b_t[:, i * 1024:(i + 1) * 1024])
        ys_pool.release()
        square_stats(b_t, st_b2)
        b_pool.release()
        rs_b_ = ln_scalars(st_b1, st_b2, NT, "ln_b")
        bv_b = fold_bias(rs_b_, W["CSWL2"][:], W["BL2"][:], "f_b")

        y1_pool = tc.alloc_tile_pool(name="y1", bufs=1, side="left")
        y1_t = y1_pool.tile([C, HW], fp32r, tag="y1_t")
        st_y1 = spool.tile([C, NT], fp32, tag="st_y1")
        st_y2 = spool.tile([C, NT], fp32, tag="st_y2")

        def b_rhs(i):
            bt_ = evt.tile([C, 1024], fp32r, tag="io1024")
            nc.sync.dma_start(bt_[:], B_SP[:, i * 1024:(i + 1) * 1024])
            return bt_

        chan_stage(b_rhs, W["WL2"][:], bv_b[:], AF.Gelu, y1_t,
                   stats=st_y1, scale=rs_b_[:, 0:1])
        square_stats(y1_t, st_y2)
        rs_y1 = ln_scalars(st_y1, st_y2, NT, "ln_y1")

        if debug:
            nc.sync.dma_start(DBG["DA"][:].bitcast(fp32r), A_SP[:])
            nc.sync.dma_start(DBG["DB"][:].bitcast(fp32r), B_SP[:])
            nc.sync.dma_start(DBG["DY1"][:].bitcast(fp32r), y1_t[:])
        # --- y tail ---
        for i in range(NT):
            my = evt.tile([C, 1024], fp32r, tag="cw1024")
            with nc.allow_low_precision(reason="fp32r mul"):
                nc.vector.tensor_mul(my[:], y1_t[:, i * 1024:(i + 1) * 1024],
                                     gx_t[:, i * 1024:(i + 1) * 1024])
            ps = ps_big.tile([C, 1024], fp32, tag="ps_big")
            for j in range(2):
                nc.tensor.matmul(ps[:, j * 512:(j + 1) * 512], W["WL3"][:],
                                 my[:, j * 512:(j + 1) * 512], start=True, stop=True)
            bt_ = evt.tile([C, 1024], fp32r, tag="io1024")
            nc.sync.dma_start(bt_[:], B_SP[:, i * 1024:(i + 1) * 1024])
            yo = evt.tile([C, 1024], fp32r, tag="cw1024")
            nc.vector.scalar_tensor_tensor(yo[:], ps[:], W["BL3"][:], bt_[:],
                                           ALU.add, ALU.add)
            nc.sync.dma_start(YO[:, i * 1024:(i + 1) * 1024].bitcast(fp32r), yo[:])
            nc.sync.dma_start(YO_SC[:, i * 1024:(i + 1) * 1024], yo[:])
        gx_pool.release()

        # --- g2 gating ---
        gy_pool = tc.alloc_tile_pool(name="gy", bufs=1, side="right")
        gy_t = gy_pool.tile([C, HW], bf16, tag="gy_t")
        v2_pool = tc.alloc_tile_pool(name="v2", bufs=1, side="right")
        v2_t = v2_pool.tile([C, HW], bf16, tag="v2_t")
        gating_w1_u(y1_t, "2", rs_y1, gy_t, v2_t)
        y1_pool.release()
        gating_v("2", gy_t, v2_t)
        if debug:
            for i in range(NT):
                dt3 = evt.tile([C, 1024], fp32, tag="cw1024")
                nc.vector.tensor_copy(dt3[:], gy_t[:, i*1024:(i+1)*1024])
                nc.sync.dma_start(DBG["DGY"][:, i*1024:(i+1)*1024], dt3[:])
        v2_pool.release()

        # --- x tail ---
        for i in range(NT):
            x1r = evt.tile([C, 1024], fp32r, tag="io1024")
            nc.sync.dma_start(x1r[:], X1_SP[:, i * 1024:(i + 1) * 1024])
            mx = evt.tile([C, 1024], fp32r, tag="cw1024")
            with nc.allow_low_precision(reason="fp32r mul"):
                nc.vector.tensor_mul(mx[:], x1r[:], gy_t[:, i * 1024:(i + 1) * 1024])
            ps = ps_big.tile([C, 1024], fp32, tag="ps_big")
            for j in range(2):
                nc.tensor.matmul(ps[:, j * 512:(j + 1) * 512], W["WL4"][:],
                                 mx[:, j * 512:(j + 1) * 512], start=True, stop=True)
            yor = evt.tile([C, 1024], fp32r, tag="io1024")
            nc.sync.dma_start(yor[:], YO_SC[:, i * 1024:(i + 1) * 1024])
            t2 = evt.tile([C, 1024], fp32r, tag="cw1024")
            nc.vector.scalar_tensor_tensor(t2[:], ps[:], W["BL4"][:], yor[:],
                                           ALU.add, ALU.add)
            ar = evt.tile([C, 1024], fp32r, tag="io1024")
            nc.sync.dma_start(ar[:], A_SP[:, i * 1024:(i + 1) * 1024])
            xo = evt.tile([C, 1024], fp32r, tag="cw1024")
            with nc.allow_low_precision(reason="fp32r add"):
                nc.vector.tensor_add(xo[:], t2[:], ar[:])
            nc.sync.dma_start(XO[:, i * 1024:(i + 1) * 1024].bitcast(fp32r), xo[:])
        gy_pool.release()

        evt.release(); spool.release(); wpool.release()
        ps_mm1.release(); ps_big.release(); ps_mm2.release()
    split_waits(nc)
    return nc


def prep_weights(inp):
    f32 = np.float32
    W = {}
    wt = np.asarray(inp["wt"], f32)
    W["WCT"] = np.ascontiguousarray(
        wt.reshape(2, 128, 128, 2, 2).transpose(1, 0, 3, 4, 2).reshape(128, 2, 4, 128))
    for k, nm in (("wc1", "WC1"), ("wc2", "WC2"), ("wl1", "WL1"),
                  ("wl2", "WL2"), ("wl3", "WL3"), ("wl4", "WL4")):
        W[nm] = np.ascontiguousarray(np.asarray(inp[k], f32))
    for g in ("1", "2"):
        w1 = np.asarray(inp[f"g{g}_w1"], f32)
        W[f"G{g}W1"] = np.ascontiguousarray(w1.reshape(128, 2, 128))
        w2 = np.asarray(inp[f"g{g}_w2"], f32)
        W[f"G{g}W2"] = np.ascontiguousarray(w2.reshape(2, 128, 256).transpose(1, 0, 2))
        w3 = np.asarray(inp[f"g{g}_w3"], f32)
        W[f"G{g}W3"] = np.ascontiguousarray(w3.reshape(2, 128, 256).transpose(1, 0, 2))
        w4 = np.asarray(inp[f"g{g}_w4"], f32)
        W[f"G{g}W4U"] = np.ascontiguousarray(w4[0:128])
        W[f"G{g}W4V"] = np.ascontiguousarray(w4[128:256])
        b2 = np.asarray(inp[f"g{g}_b2"], f32)
        b3 = np.asarray(inp[f"g{g}_b3"], f32)
        b4 = np.asarray(inp[f"g{g}_b4"], f32)
        W[f"F{g}UL"] = np.stack([w4[0:128].sum(0), b4]).astype(f32)
        W[f"F{g}UR"] = np.stack([b2, np.ones(256, f32)]).astype(f32)
        W[f"F{g}VL"] = np.ascontiguousarray(w4[128:256].sum(0)[None, :])
        W[f"F{g}VR"] = np.ascontiguousarray(b3[None, :])
        W[f"G{g}B1"] = np.ascontiguousarray(
            np.asarray(inp[f"g{g}_b1"], f32).reshape(2, 128).T)
        W[f"CSG{g}W1"] = np.ascontiguousarray(w1.sum(0).reshape(2, 128).T)
    for k, nm in (("bt", "BT"), ("bc1", "BC1"), ("bc2", "BC2"), ("bl1", "BL1"),
                  ("bl2", "BL2"), ("bl3", "BL3"), ("bl4", "BL4")):
        W[nm] = np.asarray(inp[k], f32)[:, None]
    W["CSWL1"] = np.asarray(inp["wl1"], f32).sum(0)[:, None]
    W["CSWL2"] = np.asarray(inp["wl2"], f32).sum(0)[:, None]
    W["ONESR"] = np.ones((1, 128), f32)
    W["ONESC"] = np.ones((128, 1), f32)
    return W


def make_in_maps(inp, n_cores=8):
    import ml_dtypes
    W = prep_weights(inp)
    Wc = {}
    for k, v in W.items():
        if k.endswith("W4U") or k.endswith("W4V"):
            Wc[k] = np.ascontiguousarray(v.astype(ml_dtypes.bfloat16))
        else:
            Wc[k] = np.ascontiguousarray(v.astype(np.float32))
    x = np.asarray(inp["x"], np.float32)
    y = np.asarray(inp["y"], np.float32)
    in_maps = []
    for n in range(n_cores):
        m = dict(Wc)
        m["X"] = np.ascontiguousarray(x[n].reshape(128, HW))
        m["Y"] = np.ascontiguousarray(y[n].reshape(2, 128, 4096))
        in_maps.append(m)
    return in_maps


# ======================= public entry point =======================
_NC_CACHE = {}


def _get_nc():
    if "nc" not in _NC_CACHE:
        _NC_CACHE["nc"] = build_nc()
    return _NC_CACHE["nc"]


def kernel(**inputs):
    """Full-input entry: shards batch N=8 across 8 NeuronCores (1 sample/core)."""
    from concourse.bass_utils import run_bass_kernel_spmd
    nc = _get_nc()
    in_maps = make_in_maps(inputs, n_cores=8)
    res = run_bass_kernel_spmd(nc, in_maps, core_ids=list(range(8)))
    xs, ys = [], []
    for n in range(8):
        xs.append(res.results[n]["XO"].reshape(128, 128, 128))
        ys.append(res.results[n]["YO"].reshape(128, 128, 128))
    x_out = np.stack(xs).astype(np.float32)
    y_out = np.stack(ys).astype(np.float32)
    return x_out, y_out
